# revision 74
# baseline (speedup 1.0000x reference)
"""LoRALinear kernel for Trainium2 (8 NeuronCores, SPMD data-parallel).

Computes out = x @ W.T + b + SCALE*((x@gA.T)@gB.T + (x@lA.T)@lB.T)
  x: [8, 2048, 1024] f32, W: [4096, 1024], b: [4096]
  gA/lA: [8, 1024], gB/lB: [4096, 8]  ->  out: [8, 2048, 4096] f32

Strategy: one batch of x per core. Host marshals pure layout/dtype only
(no module FLOPs): x -> x.T fp16 per core, W -> W.T fp16, b as a single
[1, 4096] f32 row (broadcast across partitions ON DEVICE via gpsimd
partition_broadcast - 16KB on the wire instead of 2MB), LoRA adapters
stacked/pre-scaled in the reference low-rank-first formulation
(A_cat = SCALE*[gA;lA] fp16, B_catT = [gB.T;lB.T] fp16, host-replicated
at partition offsets 0/32/64/96 for PE row-strip quad packing).

Device pipeline (s-phase-major, all-resident merged weights):
  wet[ot] = W.T chunk + A_cat.T @ B_catT chunk for all 8 o-tiles stays
  resident in SBUF (8MB), merged ONCE during phase A via rank-16 LoRA
  matmul QUADS (four 32-row PE strips per slot) + DVE adds scheduled so
  they never gate the PE or block the DVE FIFO. Phase A sweeps s-tiles
  0-7 across all 8 o-tiles (merges + x0/x1 DMAs land here, need-ordered
  against the input wire); phase B sweeps s-tiles 8-15 with ZERO input
  traffic - a pure matmul stream at the 216ns/slot issue floor.
  DMA discipline (trace-derived): queues are FIFO but engines run ahead
  of compute and the Tile scheduler reorders dep-free ops, so the FIFO
  order IS the only prioritizer; the shared completion-semaphore pool
  holds only ~8 in-flight DMAs, allocated at issue time with both HWDGE
  engines racing, so the first 4 DMAs per engine are exactly the
  startup-critical set (16KB per-strip slices of acat/bcatt0 that
  unblock each LoRA quad strip by ~10us even at the pre-flip half-rate
  wire); bias is a [1,4096] row broadcast on-device by gpsimd (16KB on
  the wire instead of 2MB). Out stores ride Scalar in phase A,
  alternate Scalar/Sync in phase B, and the last 4 groups fan
  quarter-stores across both queues.
Clock management (the decisive constraint): the HAM clock-gate starts
at 1.2GHz, flips to 2.4GHz only after sustained PE duty, halves again
after ~1us of PE idle, and ALSO throttles the DMA engines - so junk
N=256 warmup matmuls bridge the preamble to first-data, dep-free junk
fillers absorb DMA-arrival jitter between the prologue's kt-paced
groups, and a junk block after the last real matmul keeps the wire at
full rate while the final stores drain.
Output is stored bf16 (halves store traffic to 16.8MB/core, easing the
chip-level HBM budget shared by all 8 cores); the host converts back
to f32. fp16 operands + bf16 store give ~2e-3 relative error vs the
f32 reference (tolerance 2e-2); accumulation stays f32 in PSUM.
"""
import numpy as np
from contextlib import ExitStack

import concourse.bass as bass
import concourse.tile as tile
from concourse import bacc, mybir
from concourse.bass import ts, ds
from concourse.bass_utils import run_bass_kernel_spmd

F32 = mybir.dt.float32
F16 = mybir.dt.float16
BF16 = mybir.dt.bfloat16

N_CORES = 8
B, S, DIN, DOUT, R = 8, 2048, 1024, 4096, 8
SCALE = 16.0 / 8
R2 = 2 * R

P = 128            # partition tile
OTILE = 512        # matmul moving free dim (one PSUM bank of f32)
KT = DIN // P      # 8 k-tiles
OT = DOUT // OTILE # 8 o-tiles
ST = S // P        # 16 s-tiles
SC = S // OTILE    # 4 s-chunks of 512 for x DMA granularity
WARMUP_A = 16      # N=256 HAM warmup matmuls: a dense 100%-duty block
                   # from the engine preamble (~7.5us) past the HAM
                   # flip threshold (~10.5us), by which point the
                   # per-strip quad data (~9.5-10us) has landed
WARMUP_B = 4       # filler junks between the first quads and the first
                   # mains (DVE add-chain latency) so the PE never sees
                   # a ~1us idle (HAM would down-clock)


def build_nc():
    nc = bacc.Bacc("TRN2", target_bir_lowering=False, debug=False,
                   num_devices=N_CORES)
    xT = nc.dram_tensor("xT", [DIN, S], F16, kind="ExternalInput").ap()
    WT = nc.dram_tensor("WT", [DIN, DOUT], F16, kind="ExternalInput").ap()
    bias_row = nc.dram_tensor("bias_row", [1, DOUT], F32,
                              kind="ExternalInput").ap()
    A_cat = nc.dram_tensor("A_cat", [128, DIN], F16, kind="ExternalInput").ap()
    B_catT = nc.dram_tensor("B_catT", [128, DOUT], F16,
                            kind="ExternalInput").ap()
    # bf16 output: halves store traffic (33.5MB -> 16.8MB per core,
    # big relief on the chip-level HBM budget shared by all 8 cores)
    # for ~2e-3 relative error vs the 2e-2 tolerance; host converts
    # back to f32 (dtype-only marshalling)
    out = nc.dram_tensor("out", [S, DOUT], BF16, kind="ExternalOutput").ap()

    with tile.TileContext(nc) as tc:
        with ExitStack() as ctx:
            const = ctx.enter_context(tc.tile_pool(name="const", bufs=1))
            xt_pool = ctx.enter_context(tc.tile_pool(name="xt", bufs=1))
            wet_pool = ctx.enter_context(tc.tile_pool(name="wet", bufs=8))
            out_pool = ctx.enter_context(tc.tile_pool(name="outp", bufs=8))
            pl_pool = ctx.enter_context(tc.tile_pool(name="pl", bufs=4,
                                                     space="PSUM"))
            # two 2-deep po rings: phase A holds 2 stA banks open (poA)
            # while stB groups cycle poB; phase B alternates both
            poA_pool = ctx.enter_context(tc.tile_pool(name="poA", bufs=2,
                                                      space="PSUM"))
            poB_pool = ctx.enter_context(tc.tile_pool(name="poB", bufs=2,
                                                      space="PSUM"))

            # ---- HAM warmup ----
            junk = const.tile([P, 256], F16)
            nc.gpsimd.memset(junk[:], 1.0)

            def warmup(n, only_b=False):
                for i in range(n):
                    use_b = only_b or i % 2 == 1
                    pool = poB_pool if use_b else poA_pool
                    pw = pool.tile([P, OTILE], F32,
                                   tag="poB" if use_b else "poA")
                    nc.tensor.matmul(pw[:, 0:256], junk[:, 0:P], junk[:],
                                     start=True, stop=True)

            warmup(WARMUP_A)

            # ---- SBUF residents ----
            acat = const.tile([4 * 32, DIN], F16)
            bcatt = const.tile([4 * 32, DOUT], F16)
            bias_row_sb = const.tile([1, DOUT], F32)
            bias_full = const.tile([P, DOUT], F32)
            xts = [xt_pool.tile([P, KT, OTILE], F16, name=f"xt{sc}")
                   for sc in range(SC)]
            wets = [wet_pool.tile([P, KT, OTILE], F16, tag="wet",
                                  name=f"wet{ot}") for ot in range(OT)]

            def wet_dma(ot, k0, k1):
                src = WT[:, ts(ot, OTILE)].rearrange("(kt p) o -> p kt o",
                                                     p=P)
                nc.sync.dma_start(wets[ot][:, k0:k1, :], src[:, k0:k1, :])

            def x_dma(sc, k0, k1):
                src = xT[:, ts(sc, OTILE)].rearrange("(kt p) s -> p kt s",
                                                     p=P)
                nc.sync.dma_start(xts[sc][:, k0:k1, :], src[:, k0:k1, :])

            # ---- input DMAs. The sync queue is processed FIFO, so ONE
            # strictly need-ordered stream on sync IS the prioritizer -
            # every byte of the ~12.5MB input stream lands just before
            # its first consumer, and late items (x2/x3, wet4-7) can
            # never starve the startup-critical head. Only x0 rides the
            # scalar queue (in parallel with wet0; done by ~14.5us,
            # before the first out stores at ~20us). ----
            # DMA discipline, learned from traces: (a) DMA engines run at
            # ~half rate until the HAM flips (~10.5us); (b) the shared
            # completion-semaphore pool holds only ~8 in-flight DMAs,
            # allocated at ISSUE time with both engines racing - a 9th
            # issue blocks until a transfer completes. So the FIRST FOUR
            # DMAs on EACH engine are exactly the startup-critical set;
            # everything else queues behind recycled semaphores in
            # need-order (FIFO per queue = priority).
            def x_dma_sc(sc, k0, k1):
                src = xT[:, ts(sc, OTILE)].rearrange("(kt p) s -> p kt s",
                                                     p=P)
                nc.scalar.dma_start(xts[sc][:, k0:k1, :], src[:, k0:k1, :])

            # critical 8 (4 sync + 4 scalar): per-strip 16KB slices of
            # acat/bcatt0 - tiny transfers land ~9.5us even at the slow
            # pre-flip wire rate, so each LoRA quad strip unblocks as
            # its own strip pair arrives, and the 8 semaphores recycle
            # fast for the deferred stream
            for j in range(4):
                nc.sync.dma_start(acat[ds(32 * j, R2), ts(0, 512)],
                                  A_cat[ds(32 * j, R2), ts(0, 512)])
                nc.scalar.dma_start(bcatt[ds(32 * j, R2), ts(0, OTILE)],
                                    B_catT[ds(32 * j, R2), ts(0, OTILE)])
            # deferred stream, need-order (issues gate on sem recycling).
            # bias first: only 16KB, and its gpsimd broadcast (~3.4us)
            # must finish before the first evict (~17us)
            nc.sync.dma_start(bias_row_sb[:], bias_row)
            wet_dma(0, 0, 2)            # prologue kt-outer, kt-paced
            wet_dma(0, 2, 4)
            # acat_b before wet0's tail: the mq(0,1) quads at ~14.5us
            # block the PE FIFO on it, while wet0 kt4-7 gates only the
            # later adds/mains
            nc.sync.dma_start(acat[:, ds(512, 512)], A_cat[:, ds(512, 512)])
            wet_dma(0, 4, 8)
            x_dma(1, 0, 1)              # ot0-tail st4-7 from ~16us,
            x_dma(1, 1, 2)              # kt-paced (head slice finest)
            x_dma(1, 2, 4)
            # bcatt1 after x1's st4-critical slices: its consumer (the
            # mq(1,0) quads after st5, ~19.7us) has ~3us of slack while
            # x1(2,4) gates st4's mains at ~17us with none
            nc.sync.dma_start(bcatt[:, ts(1, OTILE)], B_catT[:, ts(1, OTILE)])
            x_dma(1, 4, 6)
            x_dma(1, 6, 8)
            wet_dma(1, 0, 4)            # mq(1,0) adds needed only ~27us
            wet_dma(1, 4, 8)
            nc.sync.dma_start(bcatt[:, ts(2, OTILE)], B_catT[:, ts(2, OTILE)])
            wet_dma(2, 0, 8)            # mq(2,*) inside ot1, ~30-33us
            nc.sync.dma_start(bcatt[:, ds(3 * OTILE, 5 * OTILE)],
                              B_catT[:, ds(3 * OTILE, 5 * OTILE)])
            wet_dma(3, 0, 8)            # mq(3,*) inside ot2, ~44-47us
            wet_dma(4, 0, 8)
            wet_dma(5, 0, 8)
            wet_dma(6, 0, 8)
            wet_dma(7, 0, 8)            # mq(7,*) inside ot6, ~95us
            x_dma(2, 0, 8)              # phase B, from ~115us
            x_dma(3, 0, 8)
            # scalar deferred stream: x0 (prologue), then stores.
            # (x1 belongs on sync: the scalar queue's ~134 B/ns behind
            # x0 delivers x1's tail at ~20.3us, later than sync's
            # ~19-20.5us - measured, twice.)
            x_dma_sc(0, 0, 2)
            x_dma_sc(0, 2, 4)
            x_dma_sc(0, 4, 8)
            # bias broadcast on idle gpsimd (needed by first evict
            # ~17us); two halves so ot0-3's chunk is ready sooner
            nc.gpsimd.partition_broadcast(bias_full[:, 0:4 * OTILE],
                                          bias_row_sb[:, 0:4 * OTILE])
            nc.gpsimd.partition_broadcast(bias_full[:, 4 * OTILE:],
                                          bias_row_sb[:, 4 * OTILE:])

            def merge_quad(ot, q):
                # wet[ot][kt] += acat.T @ bcatt chunk for kt = 4q..4q+3, as
                # four concurrently-executing rank-16 matmuls in PE row
                # strips 0-31/32-63/64-95/96-127 (one matmul slot total),
                # then four DVE adds (each gates only its own kt slice).
                kts = tuple(range(4 * q, 4 * q + 4))
                pls = [pl_pool.tile([P, OTILE], F32, tag="pl", name=f"pl{j}")
                       for j in range(4)]
                for j in range(4):
                    nc.tensor.matmul(pls[j][:],
                                     acat[ds(32 * j, R2), ts(kts[j], P)],
                                     bcatt[ds(32 * j, R2), ts(ot, OTILE)],
                                     start=True, stop=True,
                                     tile_position=(32 * j, 0))
                for j in range(4):
                    nc.vector.tensor_tensor(wets[ot][:, kts[j], :],
                                            wets[ot][:, kts[j], :], pls[j][:],
                                            mybir.AluOpType.add)

            def evict(po, st, ot, eng):
                osb = out_pool.tile([P, OTILE], BF16, tag="osb",
                                    name=f"osb{ot}_{st}")
                nc.vector.tensor_tensor(osb[:], po[:],
                                        bias_full[:, ts(ot, OTILE)],
                                        mybir.AluOpType.add)
                eng.dma_start(out[ts(st, P), ts(ot, OTILE)], osb[:])

            def mains(po, st, ot, k0, k1):
                sc, sp = st // 4, st % 4
                for kt in range(k0, k1):
                    nc.tensor.matmul(po[:], xts[sc][:, kt, ts(sp, P)],
                                     wets[ot][:, kt, :],
                                     start=(kt == 0), stop=(kt == KT - 1))

            # ---- prologue = (ot0, st0-3), kt-outer so each k-slice is
            # consumed as soon as its wet0/x0 piece lands; merge(0) quads
            # interleave. 4 held po banks (2 poA + 2 poB) + 4 pl = 8
            # PSUM. ----
            merge_quad(0, 0)
            warmup(WARMUP_B)
            pos = [poA_pool.tile([P, OTILE], F32, tag="poA", name=f"po0_{st}")
                   for st in range(2)]
            for kt in range(KT):
                if kt == 4:
                    merge_quad(0, 1)
                if kt < 6:
                    # dep-free fillers BEFORE each gated k-group (poB
                    # ring, which holds no live banks here) absorb
                    # DMA-arrival jitter so the PE never idles long
                    # enough for the HAM to down-clock
                    warmup(3, only_b=True)
                for st in range(2):
                    nc.tensor.matmul(pos[st][:], xts[0][:, kt, ts(st, P)],
                                     wets[0][:, kt, :],
                                     start=(kt == 0), stop=(kt == KT - 1))
                    if kt == KT - 1:
                        evict(pos[st], st, 0, nc.scalar)

            # ---- ot0 tail: st2-7 as full groups (st4+ on x1, kt-paced
            # as x1 lands). wet1's merges ride late (after st5/st7) so
            # their DVE adds - gated on the wet1 DMA, which follows x1
            # on the wire - never block the tail's evictions in the
            # DVE FIFO; ot1 reads wet1 only at ~27us ----
            for st in range(2, 8):
                pool, tag = ((poB_pool, "poB") if st % 2 == 0 else
                             (poA_pool, "poA"))
                po = pool.tile([P, OTILE], F32, tag=tag)
                mains(po, st, 0, 0, KT)
                evict(po, st, 0, nc.scalar)
                if st == 5:
                    merge_quad(1, 0)
                if st == 7:
                    merge_quad(1, 1)

            # ---- phase A: o-tiles 1-7 over s-tiles 0-7. Pattern per ot:
            # [stA(0-1) x kt0-3] [stB st2..st6 x kt0-7, with next-ot
            # merges after st3/st5 and paced DMA waves after st2]
            # [stA x kt4-7 + evict] [st7]. stA holds the 2 poA banks;
            # stB cycles the 2 poB banks; quads cycle 4 pl. Merge adds
            # for wet[k] complete >= 2 groups before ot_k reads them. ----
            for ot in range(1, OT):
                posA = [poA_pool.tile([P, OTILE], F32, tag="poA",
                                      name=f"poA{ot}_{st}")
                        for st in range(2)]
                for st in range(2):
                    mains(posA[st], st, ot, 0, 4)
                for st in range(2, 7):
                    po = poB_pool.tile([P, OTILE], F32, tag="poB")
                    mains(po, st, ot, 0, KT)
                    evict(po, st, ot, nc.scalar)
                    if st == 3 and ot + 1 < OT:
                        merge_quad(ot + 1, 0)
                    if st == 5 and ot + 1 < OT:
                        merge_quad(ot + 1, 1)
                for st in range(2):
                    mains(posA[st], st, ot, 4, KT)
                for st in range(2):
                    evict(posA[st], st, ot, nc.scalar)
                po = poB_pool.tile([P, OTILE], F32, tag="poB")
                mains(po, 7, ot, 0, KT)
                evict(po, 7, ot, nc.scalar)

            def evict_split(po, st, ot):
                # final-drain variant: quarter-width adds + stores
                # staggered across BOTH DGE queues so the tail drains
                # in parallel starting as early as possible
                osb = out_pool.tile([P, OTILE], BF16, tag="osb",
                                    name=f"osbs{ot}_{st}")
                for h in range(4):
                    eng = nc.scalar if h % 2 == 0 else nc.sync
                    c = ds(h * 128, 128)
                    nc.vector.tensor_tensor(osb[:, c], po[:, c],
                                            bias_full[:, ts(ot, OTILE)][:, c],
                                            mybir.AluOpType.add)
                    eng.dma_start(out[ts(st, P), ts(ot, OTILE)][:, c],
                                  osb[:, c])

            # ---- phase B: s-tiles 8-15, all inputs resident - a pure
            # matmul stream. Stores alternate Scalar/Sync so the final
            # queue drain halves; the last two groups split each store
            # across both queues. ----
            for ot in range(OT):
                for st in range(8, ST):
                    if st % 2 == 0:
                        po = poA_pool.tile([P, OTILE], F32, tag="poA")
                    else:
                        po = poB_pool.tile([P, OTILE], F32, tag="poB")
                    mains(po, st, ot, 0, KT)
                    if ot == OT - 1 and st >= ST - 4:
                        evict_split(po, st, ot)
                    else:
                        evict(po, st, ot,
                              nc.scalar if st % 2 == 0 else nc.sync)
            # drain-assist: keep the PE (and thus the HAM clock, which
            # also gates DMA-engine rate) busy while the final stores
            # drain - otherwise the clock halves and the last ~300KB
            # crawls out at half wire speed. Sized to end just after
            # the stores do (~1.8us): longer would put the junk itself
            # on the critical path to teardown.
            warmup(16)

    nc.compile()
    return nc


_NC_CACHE = None


def _get_nc():
    global _NC_CACHE
    if _NC_CACHE is None:
        _NC_CACHE = build_nc()
    return _NC_CACHE


def make_in_maps(x, W, b, global_A, global_B, local_A, local_B):
    x16 = np.asarray(x, dtype=np.float32).astype(np.float16)
    xT = np.ascontiguousarray(x16.transpose(0, 2, 1))          # [B, DIN, S]
    WT = np.ascontiguousarray(
        np.asarray(W, dtype=np.float32).T).astype(np.float16)  # [DIN, DOUT]
    bias_row = np.ascontiguousarray(
        np.asarray(b, dtype=np.float32).reshape(1, DOUT))
    a_cat = (SCALE * np.concatenate(
        [np.asarray(global_A), np.asarray(local_A)], axis=0)
    ).astype(np.float16)
    b_catT = np.concatenate(
        [np.asarray(global_B).T, np.asarray(local_B).T],
        axis=0).astype(np.float16)
    # replicate at partition offsets 0/32/64/96 for PE row-strip packing
    A_cat = np.zeros((128, DIN), dtype=np.float16)
    B_catT = np.zeros((128, DOUT), dtype=np.float16)
    for j in range(4):
        A_cat[32 * j:32 * j + R2] = a_cat
        B_catT[32 * j:32 * j + R2] = b_catT
    return [
        {"xT": xT[i], "WT": WT, "bias_row": bias_row, "A_cat": A_cat,
         "B_catT": B_catT}
        for i in range(N_CORES)
    ]


def kernel(x, W, b, global_A, global_B, local_A, local_B):
    nc = _get_nc()
    in_maps = make_in_maps(x, W, b, global_A, global_B, local_A, local_B)
    res = run_bass_kernel_spmd(nc, in_maps, list(range(N_CORES))).results
    return np.stack([np.asarray(res[i]["out"]).astype(np.float32)
                     for i in range(N_CORES)], axis=0)


# revision 75
# speedup vs baseline: 1.0286x; 1.0286x over previous
"""LoRALinear kernel for Trainium2 (8 NeuronCores, SPMD data-parallel).

Computes out = x @ W.T + b + SCALE*((x@gA.T)@gB.T + (x@lA.T)@lB.T)
  x: [8, 2048, 1024] f32, W: [4096, 1024], b: [4096]
  gA/lA: [8, 1024], gB/lB: [4096, 8]  ->  out: [8, 2048, 4096] f32

Strategy: one batch of x per core. Host marshals pure layout/dtype only
(no module FLOPs): x -> x.T fp16 per core, W -> W.T fp16, b as a single
[1, 4096] f32 row (broadcast across partitions ON DEVICE via gpsimd
partition_broadcast - 16KB on the wire instead of 2MB), LoRA adapters
stacked/pre-scaled in the reference low-rank-first formulation
(A_cat = SCALE*[gA;lA] fp16, B_catT = [gB.T;lB.T] fp16, host-replicated
at partition offsets 0/32/64/96 for PE row-strip quad packing).

Device pipeline (s-phase-major, all-resident merged weights):
  wet[ot] = W.T chunk + A_cat.T @ B_catT chunk for all 8 o-tiles stays
  resident in SBUF (8MB), merged ONCE during phase A via rank-16 LoRA
  matmul QUADS (four 32-row PE strips per slot) + DVE adds scheduled so
  they never gate the PE or block the DVE FIFO. Phase A sweeps s-tiles
  0-7 across all 8 o-tiles (merges + x0/x1 DMAs land here, need-ordered
  against the input wire); phase B sweeps s-tiles 8-15 with ZERO input
  traffic - a pure matmul stream at the 216ns/slot issue floor.
  DMA discipline (trace-derived): queues are FIFO but engines run ahead
  of compute and the Tile scheduler reorders dep-free ops, so the FIFO
  order IS the only prioritizer; the shared completion-semaphore pool
  holds only ~8 in-flight DMAs, allocated at issue time with both HWDGE
  engines racing, so the first 4 DMAs per engine are exactly the
  startup-critical set (16KB per-strip slices of acat/bcatt0 that
  unblock each LoRA quad strip by ~10us even at the pre-flip half-rate
  wire); bias is a [1,4096] row broadcast on-device by gpsimd (16KB on
  the wire instead of 2MB). Out stores ride Scalar in phase A,
  alternate Scalar/Sync in phase B, and the last 4 groups fan
  quarter-stores across both queues.
Clock management (the decisive constraint): the HAM clock-gate starts
at 1.2GHz, flips to 2.4GHz only after sustained PE duty, halves again
after ~1us of PE idle, and ALSO throttles the DMA engines - so junk
N=256 warmup matmuls bridge the preamble to first-data, dep-free junk
fillers absorb DMA-arrival jitter between the prologue's kt-paced
groups, and a junk block after the last real matmul keeps the wire at
full rate while the final stores drain.
Output is stored bf16 (halves store traffic to 16.8MB/core, easing the
chip-level HBM budget shared by all 8 cores); the host converts back
to f32. fp16 operands + bf16 store give ~2e-3 relative error vs the
f32 reference (tolerance 2e-2); accumulation stays f32 in PSUM.
"""
import numpy as np
from contextlib import ExitStack

import concourse.bass as bass
import concourse.tile as tile
from concourse import bacc, mybir
from concourse.bass import ts, ds
from concourse.bass_utils import run_bass_kernel_spmd

F32 = mybir.dt.float32
F16 = mybir.dt.float16
BF16 = mybir.dt.bfloat16

N_CORES = 8
B, S, DIN, DOUT, R = 8, 2048, 1024, 4096, 8
SCALE = 16.0 / 8
R2 = 2 * R

P = 128            # partition tile
OTILE = 512        # matmul moving free dim (one PSUM bank of f32)
KT = DIN // P      # 8 k-tiles
OT = DOUT // OTILE # 8 o-tiles
ST = S // P        # 16 s-tiles
SC = S // OTILE    # 4 s-chunks of 512 for x DMA granularity
WARMUP_A = 16      # N=256 HAM warmup matmuls: a dense 100%-duty block
                   # from the engine preamble (~7.5us) past the HAM
                   # flip threshold (~10.5us), by which point the
                   # per-strip quad data (~9.5-10us) has landed
WARMUP_B = 4       # filler junks between the first quads and the first
                   # mains (DVE add-chain latency) so the PE never sees
                   # a ~1us idle (HAM would down-clock)


def build_nc():
    nc = bacc.Bacc("TRN2", target_bir_lowering=False, debug=False,
                   num_devices=N_CORES)
    xT = nc.dram_tensor("xT", [DIN, S], F16, kind="ExternalInput").ap()
    WT = nc.dram_tensor("WT", [DIN, DOUT], F16, kind="ExternalInput").ap()
    bias_row = nc.dram_tensor("bias_row", [1, DOUT], F32,
                              kind="ExternalInput").ap()
    A_cat = nc.dram_tensor("A_cat", [128, DIN], F16, kind="ExternalInput").ap()
    B_catT = nc.dram_tensor("B_catT", [128, DOUT], F16,
                            kind="ExternalInput").ap()
    # bf16 output: halves store traffic (33.5MB -> 16.8MB per core,
    # big relief on the chip-level HBM budget shared by all 8 cores)
    # for ~2e-3 relative error vs the 2e-2 tolerance; host converts
    # back to f32 (dtype-only marshalling)
    out = nc.dram_tensor("out", [S, DOUT], BF16, kind="ExternalOutput").ap()

    with tile.TileContext(nc) as tc:
        with ExitStack() as ctx:
            const = ctx.enter_context(tc.tile_pool(name="const", bufs=1))
            xt_pool = ctx.enter_context(tc.tile_pool(name="xt", bufs=1))
            wet_pool = ctx.enter_context(tc.tile_pool(name="wet", bufs=8))
            out_pool = ctx.enter_context(tc.tile_pool(name="outp", bufs=8))
            pl_pool = ctx.enter_context(tc.tile_pool(name="pl", bufs=4,
                                                     space="PSUM"))
            # two 2-deep po rings: phase A holds 2 stA banks open (poA)
            # while stB groups cycle poB; phase B alternates both
            poA_pool = ctx.enter_context(tc.tile_pool(name="poA", bufs=2,
                                                      space="PSUM"))
            poB_pool = ctx.enter_context(tc.tile_pool(name="poB", bufs=2,
                                                      space="PSUM"))

            # ---- HAM warmup ----
            junk = const.tile([P, 256], F16)
            nc.gpsimd.memset(junk[:], 1.0)

            def warmup(n, only_b=False):
                for i in range(n):
                    use_b = only_b or i % 2 == 1
                    pool = poB_pool if use_b else poA_pool
                    pw = pool.tile([P, OTILE], F32,
                                   tag="poB" if use_b else "poA")
                    nc.tensor.matmul(pw[:, 0:256], junk[:, 0:P], junk[:],
                                     start=True, stop=True)

            warmup(WARMUP_A)

            # ---- SBUF residents ----
            acat = const.tile([4 * 32, DIN], F16)
            bcatt = const.tile([4 * 32, DOUT], F16)
            bias_row_sb = const.tile([1, DOUT], F32)
            bias_full = const.tile([P, DOUT], F32)
            xts = [xt_pool.tile([P, KT, OTILE], F16, name=f"xt{sc}")
                   for sc in range(SC)]
            wets = [wet_pool.tile([P, KT, OTILE], F16, tag="wet",
                                  name=f"wet{ot}") for ot in range(OT)]

            def wet_dma(ot, k0, k1):
                src = WT[:, ts(ot, OTILE)].rearrange("(kt p) o -> p kt o",
                                                     p=P)
                nc.sync.dma_start(wets[ot][:, k0:k1, :], src[:, k0:k1, :])

            def x_dma(sc, k0, k1):
                src = xT[:, ts(sc, OTILE)].rearrange("(kt p) s -> p kt s",
                                                     p=P)
                nc.sync.dma_start(xts[sc][:, k0:k1, :], src[:, k0:k1, :])

            # ---- input DMAs. The sync queue is processed FIFO, so ONE
            # strictly need-ordered stream on sync IS the prioritizer -
            # every byte of the ~12.5MB input stream lands just before
            # its first consumer, and late items (x2/x3, wet4-7) can
            # never starve the startup-critical head. Only x0 rides the
            # scalar queue (in parallel with wet0; done by ~14.5us,
            # before the first out stores at ~20us). ----
            # DMA discipline, learned from traces: (a) DMA engines run at
            # ~half rate until the HAM flips (~10.5us); (b) the shared
            # completion-semaphore pool holds only ~8 in-flight DMAs,
            # allocated at ISSUE time with both engines racing - a 9th
            # issue blocks until a transfer completes. So the FIRST FOUR
            # DMAs on EACH engine are exactly the startup-critical set;
            # everything else queues behind recycled semaphores in
            # need-order (FIFO per queue = priority).
            def x_dma_sc(sc, k0, k1):
                src = xT[:, ts(sc, OTILE)].rearrange("(kt p) s -> p kt s",
                                                     p=P)
                nc.scalar.dma_start(xts[sc][:, k0:k1, :], src[:, k0:k1, :])

            # critical 8 (4 sync + 4 scalar): per-strip 16KB slices of
            # acat/bcatt0 - tiny transfers land ~9.5us even at the slow
            # pre-flip wire rate, so each LoRA quad strip unblocks as
            # its own strip pair arrives, and the 8 semaphores recycle
            # fast for the deferred stream
            for j in range(4):
                nc.sync.dma_start(acat[ds(32 * j, R2), ts(0, 512)],
                                  A_cat[ds(32 * j, R2), ts(0, 512)])
                nc.scalar.dma_start(bcatt[ds(32 * j, R2), ts(0, OTILE)],
                                    B_catT[ds(32 * j, R2), ts(0, OTILE)])
            # deferred stream, need-order (issues gate on sem recycling).
            # bias first: only 16KB, and its gpsimd broadcast (~3.4us)
            # must finish before the first evict (~17us)
            nc.sync.dma_start(bias_row_sb[:], bias_row)
            wet_dma(0, 0, 2)            # prologue kt-outer, kt-paced
            wet_dma(0, 2, 4)
            # acat_b before wet0's tail: the mq(0,1) quads at ~14.5us
            # block the PE FIFO on it, while wet0 kt4-7 gates only the
            # later adds/mains
            nc.sync.dma_start(acat[:, ds(512, 512)], A_cat[:, ds(512, 512)])
            wet_dma(0, 4, 8)
            x_dma(1, 0, 1)              # ot0-tail st4-7 from ~16us,
            x_dma(1, 1, 2)              # kt-paced (head slice finest)
            nc.sync.dma_start(bcatt[:, ts(1, OTILE)], B_catT[:, ts(1, OTILE)])
            x_dma(1, 2, 4)
            x_dma(1, 4, 6)
            x_dma(1, 6, 8)
            wet_dma(1, 0, 4)            # mq(1,0) adds needed only ~27us
            wet_dma(1, 4, 8)
            nc.sync.dma_start(bcatt[:, ts(2, OTILE)], B_catT[:, ts(2, OTILE)])
            wet_dma(2, 0, 8)            # mq(2,*) inside ot1, ~30-33us
            nc.sync.dma_start(bcatt[:, ds(3 * OTILE, 5 * OTILE)],
                              B_catT[:, ds(3 * OTILE, 5 * OTILE)])
            wet_dma(3, 0, 8)            # mq(3,*) inside ot2, ~44-47us
            wet_dma(4, 0, 8)
            wet_dma(5, 0, 8)
            wet_dma(6, 0, 8)
            wet_dma(7, 0, 8)            # mq(7,*) inside ot6, ~95us
            x_dma(2, 0, 8)              # phase B, from ~115us
            x_dma(3, 0, 8)
            # scalar deferred stream: x0 (prologue), then stores.
            # (x1 belongs on sync: the scalar queue's ~134 B/ns behind
            # x0 delivers x1's tail at ~20.3us, later than sync's
            # ~19-20.5us - measured, twice.)
            x_dma_sc(0, 0, 2)
            x_dma_sc(0, 2, 4)
            x_dma_sc(0, 4, 8)
            # bias broadcast on idle gpsimd (needed by first evict
            # ~17us); two halves so ot0-3's chunk is ready sooner
            nc.gpsimd.partition_broadcast(bias_full[:, 0:4 * OTILE],
                                          bias_row_sb[:, 0:4 * OTILE])
            nc.gpsimd.partition_broadcast(bias_full[:, 4 * OTILE:],
                                          bias_row_sb[:, 4 * OTILE:])

            def merge_quad(ot, q):
                # wet[ot][kt] += acat.T @ bcatt chunk for kt = 4q..4q+3, as
                # four concurrently-executing rank-16 matmuls in PE row
                # strips 0-31/32-63/64-95/96-127 (one matmul slot total),
                # then four DVE adds (each gates only its own kt slice).
                kts = tuple(range(4 * q, 4 * q + 4))
                pls = [pl_pool.tile([P, OTILE], F32, tag="pl", name=f"pl{j}")
                       for j in range(4)]
                for j in range(4):
                    nc.tensor.matmul(pls[j][:],
                                     acat[ds(32 * j, R2), ts(kts[j], P)],
                                     bcatt[ds(32 * j, R2), ts(ot, OTILE)],
                                     start=True, stop=True,
                                     tile_position=(32 * j, 0))
                for j in range(4):
                    nc.vector.tensor_tensor(wets[ot][:, kts[j], :],
                                            wets[ot][:, kts[j], :], pls[j][:],
                                            mybir.AluOpType.add)

            def evict(po, st, ot, eng):
                osb = out_pool.tile([P, OTILE], BF16, tag="osb",
                                    name=f"osb{ot}_{st}")
                nc.vector.tensor_tensor(osb[:], po[:],
                                        bias_full[:, ts(ot, OTILE)],
                                        mybir.AluOpType.add)
                eng.dma_start(out[ts(st, P), ts(ot, OTILE)], osb[:])

            def mains(po, st, ot, k0, k1):
                sc, sp = st // 4, st % 4
                for kt in range(k0, k1):
                    nc.tensor.matmul(po[:], xts[sc][:, kt, ts(sp, P)],
                                     wets[ot][:, kt, :],
                                     start=(kt == 0), stop=(kt == KT - 1))

            # ---- prologue = (ot0, st0-3), kt-outer so each k-slice is
            # consumed as soon as its wet0/x0 piece lands; merge(0) quads
            # interleave. 4 held po banks (2 poA + 2 poB) + 4 pl = 8
            # PSUM. ----
            merge_quad(0, 0)
            warmup(WARMUP_B)
            pos = [poA_pool.tile([P, OTILE], F32, tag="poA", name=f"po0_{st}")
                   for st in range(2)]
            for kt in range(KT):
                if kt == 4:
                    merge_quad(0, 1)
                if kt < 6:
                    # dep-free fillers BEFORE each gated k-group (poB
                    # ring, which holds no live banks here) absorb
                    # DMA-arrival jitter so the PE never idles long
                    # enough for the HAM to down-clock
                    warmup(3, only_b=True)
                for st in range(2):
                    nc.tensor.matmul(pos[st][:], xts[0][:, kt, ts(st, P)],
                                     wets[0][:, kt, :],
                                     start=(kt == 0), stop=(kt == KT - 1))
                    if kt == KT - 1:
                        evict(pos[st], st, 0, nc.scalar)

            # ---- ot0 tail: st2-7 as full groups (st4+ on x1, kt-paced
            # as x1 lands). wet1's merges ride late (after st5/st7) so
            # their DVE adds - gated on the wet1 DMA, which follows x1
            # on the wire - never block the tail's evictions in the
            # DVE FIFO; ot1 reads wet1 only at ~27us ----
            for st in range(2, 8):
                pool, tag = ((poB_pool, "poB") if st % 2 == 0 else
                             (poA_pool, "poA"))
                po = pool.tile([P, OTILE], F32, tag=tag)
                mains(po, st, 0, 0, KT)
                evict(po, st, 0, nc.scalar)
                if st == 5:
                    merge_quad(1, 0)
                if st == 7:
                    merge_quad(1, 1)

            # ---- phase A: o-tiles 1-7 over s-tiles 0-7. Pattern per ot:
            # [stA(0-1) x kt0-3] [stB st2..st6 x kt0-7, with next-ot
            # merges after st3/st5 and paced DMA waves after st2]
            # [stA x kt4-7 + evict] [st7]. stA holds the 2 poA banks;
            # stB cycles the 2 poB banks; quads cycle 4 pl. Merge adds
            # for wet[k] complete >= 2 groups before ot_k reads them. ----
            for ot in range(1, OT):
                posA = [poA_pool.tile([P, OTILE], F32, tag="poA",
                                      name=f"poA{ot}_{st}")
                        for st in range(2)]
                for st in range(2):
                    mains(posA[st], st, ot, 0, 4)
                for st in range(2, 7):
                    po = poB_pool.tile([P, OTILE], F32, tag="poB")
                    mains(po, st, ot, 0, KT)
                    evict(po, st, ot, nc.scalar)
                    if st == 3 and ot + 1 < OT:
                        merge_quad(ot + 1, 0)
                    if st == 5 and ot + 1 < OT:
                        merge_quad(ot + 1, 1)
                for st in range(2):
                    mains(posA[st], st, ot, 4, KT)
                for st in range(2):
                    evict(posA[st], st, ot, nc.scalar)
                po = poB_pool.tile([P, OTILE], F32, tag="poB")
                mains(po, 7, ot, 0, KT)
                evict(po, 7, ot, nc.scalar)

            def evict_split(po, st, ot):
                # final-drain variant: quarter-width adds + stores
                # staggered across BOTH DGE queues so the tail drains
                # in parallel starting as early as possible
                osb = out_pool.tile([P, OTILE], BF16, tag="osb",
                                    name=f"osbs{ot}_{st}")
                for h in range(4):
                    eng = nc.scalar if h % 2 == 0 else nc.sync
                    c = ds(h * 128, 128)
                    nc.vector.tensor_tensor(osb[:, c], po[:, c],
                                            bias_full[:, ts(ot, OTILE)][:, c],
                                            mybir.AluOpType.add)
                    eng.dma_start(out[ts(st, P), ts(ot, OTILE)][:, c],
                                  osb[:, c])

            # ---- phase B: s-tiles 8-15, all inputs resident - a pure
            # matmul stream. Stores alternate Scalar/Sync so the final
            # queue drain halves; the last two groups split each store
            # across both queues. ----
            for ot in range(OT):
                for st in range(8, ST):
                    if st % 2 == 0:
                        po = poA_pool.tile([P, OTILE], F32, tag="poA")
                    else:
                        po = poB_pool.tile([P, OTILE], F32, tag="poB")
                    mains(po, st, ot, 0, KT)
                    if ot == OT - 1 and st >= ST - 4:
                        evict_split(po, st, ot)
                    else:
                        evict(po, st, ot,
                              nc.scalar if st % 2 == 0 else nc.sync)
            # drain-assist: keep the PE (and thus the HAM clock, which
            # also gates DMA-engine rate) busy while the final stores
            # drain - otherwise the clock halves and the last ~300KB
            # crawls out at half wire speed. Sized to end just after
            # the stores do (~1.8us): longer would put the junk itself
            # on the critical path to teardown.
            warmup(16)

    nc.compile()
    return nc


_NC_CACHE = None


def _get_nc():
    global _NC_CACHE
    if _NC_CACHE is None:
        _NC_CACHE = build_nc()
    return _NC_CACHE


def make_in_maps(x, W, b, global_A, global_B, local_A, local_B):
    x16 = np.asarray(x, dtype=np.float32).astype(np.float16)
    xT = np.ascontiguousarray(x16.transpose(0, 2, 1))          # [B, DIN, S]
    WT = np.ascontiguousarray(
        np.asarray(W, dtype=np.float32).T).astype(np.float16)  # [DIN, DOUT]
    bias_row = np.ascontiguousarray(
        np.asarray(b, dtype=np.float32).reshape(1, DOUT))
    a_cat = (SCALE * np.concatenate(
        [np.asarray(global_A), np.asarray(local_A)], axis=0)
    ).astype(np.float16)
    b_catT = np.concatenate(
        [np.asarray(global_B).T, np.asarray(local_B).T],
        axis=0).astype(np.float16)
    # replicate at partition offsets 0/32/64/96 for PE row-strip packing
    A_cat = np.zeros((128, DIN), dtype=np.float16)
    B_catT = np.zeros((128, DOUT), dtype=np.float16)
    for j in range(4):
        A_cat[32 * j:32 * j + R2] = a_cat
        B_catT[32 * j:32 * j + R2] = b_catT
    return [
        {"xT": xT[i], "WT": WT, "bias_row": bias_row, "A_cat": A_cat,
         "B_catT": B_catT}
        for i in range(N_CORES)
    ]


def kernel(x, W, b, global_A, global_B, local_A, local_B):
    nc = _get_nc()
    in_maps = make_in_maps(x, W, b, global_A, global_B, local_A, local_B)
    res = run_bass_kernel_spmd(nc, in_maps, list(range(N_CORES))).results
    return np.stack([np.asarray(res[i]["out"]).astype(np.float32)
                     for i in range(N_CORES)], axis=0)
